# revision 3
# baseline (speedup 1.0000x reference)
"""Trainium2 Bass kernel for nn_Dense_BinaryLayer (binary-weight dense layer).

out = x @ Wb + b, where Wb = binarize(W) in {-1, +1}.

Strategy: data-parallel over the 8 NeuronCores — each core handles 2048 rows
of x and the full (replicated) W and b; no collectives.  Host-side prep is
pure data movement: each core's x slice is transposed to k-major layout and
both x and W are passed as the high 16 bits of each f32 (a byte-slice view =
bf16 truncation; no arithmetic).  Binarization on truncated W is exact (W
values are multiples of ~2^-22, so no value crosses the >2^-24 threshold
under truncation), and bf16-truncated x gives rel err ~3e-3 vs the 2e-2
gate (verified empirically).

This halves HBM traffic vs f32 x/W (14 MiB vs 20 MiB per core) and removes
all on-chip dtype conversion of x.  Per core:
  - W streams as 8 per-k-tile chunks on the Activation HWDGE ring (earliest
    available, dedicated to W); DVE binarizes each chunk as it lands
    (bf16 2x rate): m = (W > 2^-24) in {0,1}, Wb = 2m-1 in {+-1}.
  - x row-tiles 0 and 1 arrive first on the SP HWDGE ring; the remaining
    7 row-pairs stream via SWDGE (gpsimd).
  - bf16 matmuls (full PE rate at free dim 512) accumulate in PSUM over the
    8 k-tiles; DVE adds the broadcast bias while evicting; per-row-tile
    stores rotate across the three DMA rings.
"""
import sys

sys.path.insert(0, "/opt/trn_rl_repo")

import numpy as np

N_TOTAL = 16384
D_IN = 1024
D_OUT = 1024
N_CORES = 8
ROWS = N_TOTAL // N_CORES      # 2048 rows per core
P = 128
K_TILES = D_IN // P            # 8
I_TILES = ROWS // P            # 16
PAIRS = I_TILES // 2           # 8
BIN_THRESH = 2.0 ** -24

_cached = {}


def _build():
    import concourse.tile as tile
    from concourse import bacc, mybir

    f32 = mybir.dt.float32
    bf16 = mybir.dt.bfloat16
    TS = mybir.AluOpType

    nc = bacc.Bacc()
    xt_d = nc.declare_dram_parameter("xT", [D_IN, ROWS], bf16, isOutput=False)
    w_d = nc.declare_dram_parameter("W", [D_IN, D_OUT], bf16, isOutput=False)
    b_d = nc.declare_dram_parameter("b", [D_OUT], f32, isOutput=False)
    o_d = nc.declare_dram_parameter("out", [ROWS, D_OUT], f32, isOutput=True)

    with tile.TileContext(nc) as tc:
        with (
            tc.tile_pool(name="const", bufs=1) as const,
            tc.tile_pool(name="wpool", bufs=1) as wpool,
            tc.tile_pool(name="xts", bufs=4) as xts,
            tc.tile_pool(name="outp", bufs=4) as outp,
            tc.tile_pool(name="pso", bufs=3, space="PSUM") as pso,
        ):
            xt_ap = xt_d[:].rearrange("(kt p) i -> p kt i", p=P)
            w_ap = w_d[:].rearrange("(kt p) j -> p kt j", p=P)

            # first two x row-tiles individually on the SP HWDGE ring
            # (earliest data on chip), so matmul 0 starts ASAP
            x0 = const.tile([P, K_TILES, P], bf16, tag="x0")
            nc.sync.dma_start(x0[:], xt_ap[:, :, 0:128])
            x1 = const.tile([P, K_TILES, P], bf16, tag="x1")
            nc.sync.dma_start(x1[:], xt_ap[:, :, 128:256])

            # W as 8 per-k-tile chunks on the Act HWDGE ring so wb[kt]
            # becomes ready incrementally from ~4us
            w_raw = wpool.tile([P, K_TILES, D_OUT], bf16, tag="wraw")
            for kt in range(K_TILES):
                nc.scalar.dma_start(w_raw[:, kt, :], w_ap[:, kt, :])

            # bias broadcast to all partitions
            bb = const.tile([P, D_OUT], f32, tag="bb")
            nc.sync.dma_start(bb[:], b_d[:].unsqueeze(0).partition_broadcast(P))

            # x row-pairs 1..7 via SWDGE
            x_pairs = [None] * PAIRS
            for pr in range(1, PAIRS):
                t = xts.tile([P, K_TILES, 256], bf16, tag="x", name=f"xt_{pr}")
                nc.gpsimd.dma_start(t[:], xt_ap[:, :, pr * 256:(pr + 1) * 256])
                x_pairs[pr] = t

            # binarize on DVE per k-tile (bf16 => 2x DVE rate):
            # m = (W > c) in {0,1}, then Wb = 2m-1 in {+-1}
            wb = wpool.tile([P, K_TILES, D_OUT], bf16, tag="wb")
            wm = wpool.tile([P, D_OUT], bf16, tag="wm")
            for kt in range(K_TILES):
                nc.vector.tensor_scalar(
                    wm[:], w_raw[:, kt, :], BIN_THRESH, None, TS.is_gt,
                )
                nc.vector.tensor_scalar(
                    wb[:, kt, :], wm[:], 2.0, 1.0, TS.mult, TS.subtract,
                )

            def x_tile(it):
                if it == 0:
                    return x0[:]
                if it == 1:
                    return x1[:]
                pr, half = divmod(it, 2)
                return x_pairs[pr][:, :, half * P:(half + 1) * P]

            for it in range(I_TILES):
                src = x_tile(it)
                ps_o = pso.tile([P, D_OUT], f32, tag="pso", name=f"pso_{it}")
                for kt in range(K_TILES):
                    first = kt == 0
                    last = kt == K_TILES - 1
                    nc.tensor.matmul(
                        ps_o[:, 0:512],
                        src[:, kt, :],
                        wb[:, kt, 0:512],
                        start=first, stop=last,
                    )
                    nc.tensor.matmul(
                        ps_o[:, 512:1024],
                        src[:, kt, :],
                        wb[:, kt, 512:1024],
                        start=first, stop=last,
                    )
                out_sb = outp.tile([P, D_OUT], f32, tag="out", name=f"out_{it}")
                nc.vector.tensor_tensor(
                    out=out_sb[:], in0=ps_o[:], in1=bb[:], op=TS.add,
                )
                ring = (nc.sync, nc.scalar, nc.gpsimd)[it % 3]
                ring.dma_start(o_d[it * P:(it + 1) * P, :], out_sb[:])

    nc.compile()
    nc.finalize()
    return nc


def _hi16(a):
    """bf16 truncation of a C-contiguous f32 array as a byte-slice view."""
    import ml_dtypes

    u = a.view(np.uint16).reshape(*a.shape, 2)[..., 1]
    return np.ascontiguousarray(u).view(ml_dtypes.bfloat16)


def make_in_maps(x, W, b):
    x = np.ascontiguousarray(np.asarray(x, dtype=np.float32))
    W = np.ascontiguousarray(np.asarray(W, dtype=np.float32))
    b = np.ascontiguousarray(np.asarray(b, dtype=np.float32))
    W16 = _hi16(W)
    return [
        {
            "xT": _hi16(np.ascontiguousarray(x[c * ROWS:(c + 1) * ROWS].T)),
            "W": W16,
            "b": b,
        }
        for c in range(N_CORES)
    ]


def kernel(x, W, b):
    from concourse.bass_utils import run_bass_kernel_spmd

    if "nc" not in _cached:
        _cached["nc"] = _build()
    nc = _cached["nc"]

    in_maps = make_in_maps(x, W, b)
    res = run_bass_kernel_spmd(nc, in_maps, list(range(N_CORES)))
    out = np.concatenate([res.results[c]["out"] for c in range(N_CORES)], axis=0)
    return out.astype(np.float32, copy=False)


# revision 4
# speedup vs baseline: 1.0534x; 1.0534x over previous
"""Trainium2 Bass kernel for nn_Dense_BinaryLayer (binary-weight dense layer).

out = x @ Wb + b, where Wb = binarize(W) in {-1, +1}.

Strategy: data-parallel over the 8 NeuronCores — each core handles 2048 rows
of x and the full (replicated) W and b; no collectives.  Host-side prep is
pure data movement: each core's x slice is permuted into per-row-tile
k-major blocks ([it, p, kt, r] layout, so every DMA is one contiguous
256 KiB DRAM read with 2 KiB per-partition segments), and both x and W are
passed as the high 16 bits of each f32 (byte-slice view = bf16 truncation,
no arithmetic).  Binarization on truncated W is exact (W values are
multiples of ~2^-22, so truncation cannot cross the >2^-24 threshold), and
bf16-truncated x gives rel err ~3e-3 vs the 2e-2 gate (verified).

Per core (HBM traffic 14.2 MiB vs 20.9 for the f32 version):
  - W streams as 8 per-k-tile chunks (contiguous 256 KiB rows of the bf16
    array); DVE binarizes each chunk as it lands (bf16 2x DVE rate).
  - x row-tiles stream in consumption order: it0/W0/it1/W1 on the SP HWDGE
    ring, it2/it3 + remaining W on the Act HWDGE ring, it4..15 via SWDGE.
    All x tiles get dedicated SBUF buffers (no recycling stalls).
  - bf16 matmuls (full PE rate, free dim 512) accumulate in PSUM over the
    8 k-tiles (4 PSUM tiles in flight); DVE adds the broadcast bias on
    eviction; per-row-tile stores rotate across the three DMA rings.
"""
import sys

sys.path.insert(0, "/opt/trn_rl_repo")

import numpy as np

N_TOTAL = 16384
D_IN = 1024
D_OUT = 1024
N_CORES = 8
ROWS = N_TOTAL // N_CORES      # 2048 rows per core
P = 128
K_TILES = D_IN // P            # 8
I_TILES = ROWS // P            # 16
BIN_THRESH = 2.0 ** -24

_cached = {}


def _build():
    import concourse.tile as tile
    from concourse import bacc, mybir

    f32 = mybir.dt.float32
    bf16 = mybir.dt.bfloat16
    TS = mybir.AluOpType

    nc = bacc.Bacc()
    # x in [it, p, kt, r] block layout (host-permuted): row-tile it is one
    # contiguous 256 KiB block, 2 KiB contiguous per partition
    xt_d = nc.declare_dram_parameter(
        "xT", [I_TILES * P, K_TILES * P], bf16, isOutput=False)
    w_d = nc.declare_dram_parameter("W", [D_IN, D_OUT], bf16, isOutput=False)
    b_d = nc.declare_dram_parameter("b", [D_OUT], f32, isOutput=False)
    o_d = nc.declare_dram_parameter("out", [ROWS, D_OUT], f32, isOutput=True)

    with tile.TileContext(nc) as tc:
        with (
            tc.tile_pool(name="const", bufs=1) as const,
            tc.tile_pool(name="wpool", bufs=1) as wpool,
            tc.tile_pool(name="xts", bufs=I_TILES) as xts,
            tc.tile_pool(name="outp", bufs=8) as outp,
            tc.tile_pool(name="pso", bufs=4, space="PSUM") as pso,
        ):
            xt_ap = xt_d[:].rearrange("(it p) (kt r) -> it p kt r", p=P, kt=K_TILES)
            w_ap = w_d[:].rearrange("(kt p) j -> p kt j", p=P)

            w_raw = wpool.tile([P, K_TILES, D_OUT], bf16, tag="wraw")
            x_tiles = []
            for it in range(I_TILES):
                x_tiles.append(
                    xts.tile([P, K_TILES, P], bf16, tag="x", name=f"xt_{it}"))

            # interleave first x tiles and W chunks across the two HWDGE
            # rings in consumption order; bulk x via SWDGE
            nc.sync.dma_start(x_tiles[0][:], xt_ap[0])
            nc.scalar.dma_start(w_raw[:, 0, :], w_ap[:, 0, :])
            nc.sync.dma_start(x_tiles[1][:], xt_ap[1])
            nc.scalar.dma_start(w_raw[:, 1, :], w_ap[:, 1, :])
            nc.sync.dma_start(x_tiles[2][:], xt_ap[2])
            nc.scalar.dma_start(w_raw[:, 2, :], w_ap[:, 2, :])
            nc.sync.dma_start(x_tiles[3][:], xt_ap[3])
            for kt in range(3, K_TILES):
                nc.scalar.dma_start(w_raw[:, kt, :], w_ap[:, kt, :])

            # bias broadcast to all partitions
            bb = const.tile([P, D_OUT], f32, tag="bb")
            nc.sync.dma_start(bb[:], b_d[:].unsqueeze(0).partition_broadcast(P))

            for it in range(4, I_TILES):
                nc.gpsimd.dma_start(x_tiles[it][:], xt_ap[it])

            # binarize on DVE per k-tile (bf16 => 2x DVE rate):
            # m = (W > c) in {0,1}, then Wb = 2m-1 in {+-1}
            wb = wpool.tile([P, K_TILES, D_OUT], bf16, tag="wb")
            wm = wpool.tile([P, D_OUT], bf16, tag="wm")
            for kt in range(K_TILES):
                nc.vector.tensor_scalar(
                    wm[:], w_raw[:, kt, :], BIN_THRESH, None, TS.is_gt,
                )
                nc.vector.tensor_scalar(
                    wb[:, kt, :], wm[:], 2.0, 1.0, TS.mult, TS.subtract,
                )

            for it in range(I_TILES):
                src = x_tiles[it]
                ps_o = pso.tile([P, D_OUT], f32, tag="pso", name=f"pso_{it}")
                for kt in range(K_TILES):
                    first = kt == 0
                    last = kt == K_TILES - 1
                    nc.tensor.matmul(
                        ps_o[:, 0:512],
                        src[:, kt, :],
                        wb[:, kt, 0:512],
                        start=first, stop=last,
                    )
                    nc.tensor.matmul(
                        ps_o[:, 512:1024],
                        src[:, kt, :],
                        wb[:, kt, 512:1024],
                        start=first, stop=last,
                    )
                out_sb = outp.tile([P, D_OUT], f32, tag="out", name=f"out_{it}")
                nc.vector.tensor_tensor(
                    out=out_sb[:], in0=ps_o[:], in1=bb[:], op=TS.add,
                )
                ring = (nc.sync, nc.scalar, nc.gpsimd)[it % 3]
                ring.dma_start(o_d[it * P:(it + 1) * P, :], out_sb[:])

    nc.compile()
    nc.finalize()
    return nc


def _hi16(a):
    """bf16 truncation of a C-contiguous f32 array as a byte-slice view."""
    import ml_dtypes

    u = a.view(np.uint16).reshape(*a.shape, 2)[..., 1]
    return np.ascontiguousarray(u).view(ml_dtypes.bfloat16)


def make_in_maps(x, W, b):
    x = np.ascontiguousarray(np.asarray(x, dtype=np.float32))
    W = np.ascontiguousarray(np.asarray(W, dtype=np.float32))
    b = np.ascontiguousarray(np.asarray(b, dtype=np.float32))
    W16 = _hi16(W)
    maps = []
    for c in range(N_CORES):
        xc = x[c * ROWS:(c + 1) * ROWS]
        # [it, r, kt, p] -> [it, p, kt, r]: row-tile blocks, k-major inside
        blk = np.ascontiguousarray(
            xc.reshape(I_TILES, P, K_TILES, P).transpose(0, 3, 2, 1))
        maps.append({
            "xT": _hi16(blk).reshape(I_TILES * P, K_TILES * P),
            "W": W16,
            "b": b,
        })
    return maps


def kernel(x, W, b):
    from concourse.bass_utils import run_bass_kernel_spmd

    if "nc" not in _cached:
        _cached["nc"] = _build()
    nc = _cached["nc"]

    in_maps = make_in_maps(x, W, b)
    res = run_bass_kernel_spmd(nc, in_maps, list(range(N_CORES)))
    out = np.concatenate([res.results[c]["out"] for c in range(N_CORES)], axis=0)
    return out.astype(np.float32, copy=False)
